# revision 1
# baseline (speedup 1.0000x reference)
"""Trainium2 Bass kernel for nn_ComposedFeatureTransformer (embedding lookup).

out_s[b, :] = bias + sum_k values_s[b, k] * merged_weight[indices_s[b, k], :]
for s in {0, 1}.

Strategy: data-parallel over the batch dim across 8 NeuronCores (512 rows
per core, both feature sets on every core). The 186 MB weight table stays
in each core's HBM; rows are fetched with indirect (gather) DMA, 128 rows
per instruction. ACT applies the per-(batch,k) value scale, DVE
accumulates, bias is folded into the k=0 accumulation.
"""

import numpy as np

import concourse.bacc as bacc
import concourse.bass as bass
import concourse.mybir as mybir
import concourse.tile as tile
from concourse.bass_utils import run_bass_kernel_spmd

N_CORES = 8
BATCH = 4096
PER_CORE = BATCH // N_CORES  # 512
K = 32
V = 45056
D = 1032
P = 128
N_TILES = PER_CORE // P  # 4

TRACE = False  # set by test harness to collect an NTFF profile
LAST_RESULT = None  # BassKernelResults of the last run (for profiling)

_NC = None


def _build():
    nc = bacc.Bacc("TRN2", debug=False, num_devices=N_CORES)
    f32 = mybir.dt.float32
    i32 = mybir.dt.int32

    idx_d = [
        nc.dram_tensor(f"idx{s}", [PER_CORE, K], i32, kind="ExternalInput")
        for s in range(2)
    ]
    val_d = [
        nc.dram_tensor(f"val{s}", [PER_CORE, K], f32, kind="ExternalInput")
        for s in range(2)
    ]
    w_d = nc.dram_tensor("weight", [V, D], f32, kind="ExternalInput")
    bias_d = nc.dram_tensor("bias_rep", [P, D], f32, kind="ExternalInput")
    out_d = [
        nc.dram_tensor(f"out{s}", [PER_CORE, D], f32, kind="ExternalOutput")
        for s in range(2)
    ]

    with tile.TileContext(nc) as tc:
        with (
            tc.tile_pool(name="const", bufs=1) as const_pool,
            tc.tile_pool(name="io", bufs=2) as io_pool,
            tc.tile_pool(name="rows", bufs=8) as rows_pool,
            tc.tile_pool(name="acc", bufs=2) as acc_pool,
        ):
            bias_sb = const_pool.tile([P, D], f32)
            nc.sync.dma_start(out=bias_sb[:], in_=bias_d[:])
            for s in range(2):
                for t in range(N_TILES):
                    rs = slice(t * P, (t + 1) * P)
                    idx_sb = io_pool.tile([P, K], i32, tag="idx")
                    val_sb = io_pool.tile([P, K], f32, tag="val")
                    nc.sync.dma_start(out=idx_sb[:], in_=idx_d[s][rs, :])
                    nc.sync.dma_start(out=val_sb[:], in_=val_d[s][rs, :])
                    acc = acc_pool.tile([P, D], f32, tag="acc")
                    for k in range(K):
                        rows = rows_pool.tile([P, D], f32, tag="rows")
                        nc.gpsimd.indirect_dma_start(
                            out=rows[:],
                            out_offset=None,
                            in_=w_d[:],
                            in_offset=bass.IndirectOffsetOnAxis(
                                ap=idx_sb[:, k : k + 1], axis=0
                            ),
                        )
                        scaled = rows_pool.tile([P, D], f32, tag="scaled")
                        nc.scalar.activation(
                            out=scaled[:],
                            in_=rows[:],
                            func=mybir.ActivationFunctionType.Copy,
                            scale=val_sb[:, k : k + 1],
                        )
                        if k == 0:
                            nc.vector.tensor_add(
                                out=acc[:], in0=scaled[:], in1=bias_sb[:]
                            )
                        else:
                            nc.vector.tensor_add(out=acc[:], in0=acc[:], in1=scaled[:])
                    nc.sync.dma_start(out=out_d[s][rs, :], in_=acc[:])

    nc.compile()
    return nc


def _get_nc():
    global _NC
    if _NC is None:
        _NC = _build()
    return _NC


def kernel(
    feature_indices_0,
    feature_values_0,
    feature_indices_1,
    feature_values_1,
    merged_weight,
    bias,
):
    global LAST_RESULT
    idx0 = np.ascontiguousarray(np.asarray(feature_indices_0, dtype=np.int32))
    idx1 = np.ascontiguousarray(np.asarray(feature_indices_1, dtype=np.int32))
    val0 = np.ascontiguousarray(np.asarray(feature_values_0, dtype=np.float32))
    val1 = np.ascontiguousarray(np.asarray(feature_values_1, dtype=np.float32))
    w = np.ascontiguousarray(np.asarray(merged_weight, dtype=np.float32))
    b = np.asarray(bias, dtype=np.float32)
    bias_rep = np.ascontiguousarray(np.broadcast_to(b[None, :], (P, D)))

    nc = _get_nc()
    in_maps = []
    for c in range(N_CORES):
        rs = slice(c * PER_CORE, (c + 1) * PER_CORE)
        in_maps.append(
            {
                "idx0": idx0[rs],
                "val0": val0[rs],
                "idx1": idx1[rs],
                "val1": val1[rs],
                "weight": w,
                "bias_rep": bias_rep,
            }
        )

    res = run_bass_kernel_spmd(
        nc, in_maps, core_ids=list(range(N_CORES)), trace=TRACE
    )
    LAST_RESULT = res
    out0 = np.concatenate([res.results[c]["out0"] for c in range(N_CORES)], axis=0)
    out1 = np.concatenate([res.results[c]["out1"] for c in range(N_CORES)], axis=0)
    return out0, out1
